# revision 21
# baseline (speedup 1.0000x reference)
"""Local Gaussian refinement kernel for Trainium2 (8 NeuronCores, SPMD).

For each (b, k): round+clip the coarse coordinate, gather the 5x5 patch of
the heatmap around it, masked softmax over the 25 logits, return the
softmax-weighted expected (x, y).

Strategy: the op only touches 25 floats of each 192x256 heatmap slice, so
instead of streaming the full 428 MB array we do an *indirect DMA gather*:
the device computes, from the coords alone, a flat element offset for each
of the 5 patch rows of each pair, and one indirect DMA fetches all
272 pairs x 5 rows x 5 contiguous floats per core.  Everything else
(rounding, clipping, masks, softmax, expectation) also runs on device.

Sharding: data-parallel over batch; core m gets batches [16m, 16m+16).
272 (b,k) pairs per core are laid out as pair g = p + 128*t with
p in [0,128) partitions and t in {0,1,2} free-dim chunks (pairs 272..383
are padding, clamped + discarded).
"""

import sys

sys.path.insert(0, "/opt/trn_rl_repo")

import numpy as np

import concourse.bass as bass
import concourse.bacc as bacc
import concourse.tile as tile
from concourse import mybir
from concourse.bass_utils import run_bass_kernel_spmd

# Problem constants (hardcoded per contract).
B, K, H, W = 128, 17, 192, 256
NCORES = 8
BS = B // NCORES  # 16 batches per core
PAIRS = BS * K  # 272 (b,k) pairs per core
P = 128  # SBUF partitions
T = 3  # ceil(PAIRS / P) free-dim chunks
PADP = P * T  # 384 padded pairs
R = BS * K * H  # 52224 heatmap rows per core
NELEM = R * W  # 13369344 f32 elements per core shard
WN = 5  # window size (2*r+1)
BIGF = float(2 ** 23)  # RNE rounding trick constant
NEGM = 50.0  # additive mask magnitude (exp(-44) ~ 8e-20, exact enough)
F32 = mybir.dt.float32
I32 = mybir.dt.int32
A = mybir.AluOpType


def _bcast_col(ap, t, n):
    """View column t of a [128, C] AP as [128, n] with 0-stride broadcast."""
    return bass.AP(ap.tensor, ap.offset + t, [ap.ap[0], [0, n]])


def build_program():
    # Bacc (not plain Bass): its compile() runs generate_event_semaphores,
    # which splits instructions with >1 semaphore wait (TRN2 HW limit).
    nc = bacc.Bacc(None, target_bir_lowering=False)
    heat = nc.dram_tensor("heat", [R, W], F32, kind="ExternalInput")
    coords = nc.dram_tensor("coords", [PADP, 2], F32, kind="ExternalInput")
    out = nc.dram_tensor("out", [PADP, 2], F32, kind="ExternalOutput")

    with tile.TileContext(nc) as tc:
        with tc.tile_pool(name="sb", bufs=1) as pool:
            # ---- constants (iota) -------------------------------------
            goff_i = pool.tile([P, T], I32)  # pair id g = p + 128t
            nc.gpsimd.iota(goff_i[:], [[P, T]], base=0, channel_multiplier=1)
            goff = pool.tile([P, T], F32)  # g * H*W (exact: < 2^24)
            nc.vector.tensor_copy(goff[:], goff_i[:])
            nc.vector.tensor_scalar(goff[:], goff[:], float(H * W), None, A.mult)

            xoff_i = pool.tile([P, T * WN * WN], I32)  # value = j over (t,i,j)
            nc.gpsimd.iota(
                xoff_i[:], [[0, T], [0, WN], [1, WN]], base=0, channel_multiplier=0
            )
            xoff = pool.tile([P, T * WN * WN], F32)
            nc.vector.tensor_copy(xoff[:], xoff_i[:])

            yoff_i = pool.tile([P, T * WN * WN], I32)  # value = i over (t,i,j)
            nc.gpsimd.iota(
                yoff_i[:], [[0, T], [1, WN], [0, WN]], base=0, channel_multiplier=0
            )
            yoff = pool.tile([P, T * WN * WN], F32)
            nc.vector.tensor_copy(yoff[:], yoff_i[:])

            # ---- load coords ------------------------------------------
            crd = pool.tile([P, T * 2], F32)  # [p, (t,c)]
            nc.sync.dma_start(
                out=crd[:], in_=coords[:, :].rearrange("(t p) c -> p t c", p=P)
            )

            # ---- round-half-even + window bases -----------------------
            # (x + 2^23) then (- 2^23): two separate instructions so each
            # result is rounded to fp32 => exact round-to-nearest-even.
            # Done on the whole [128,6] coords tile (x and y together).
            pxy = pool.tile([P, T * 2], F32)
            nc.vector.tensor_scalar(pxy[:], crd[:], BIGF, None, A.add)
            nc.vector.tensor_scalar(pxy[:], pxy[:], BIGF, None, A.subtract)
            px = bass.AP(pxy[:].tensor, pxy[:].offset, [pxy[:].ap[0], [2, T]])
            py = bass.AP(pxy[:].tensor, pxy[:].offset + 1, [pxy[:].ap[0], [2, T]])

            cbase = pool.tile([P, T], F32)  # clip(px-2, 0, W-5)
            nc.vector.tensor_scalar(cbase[:], px, 2.0, 0.0, A.subtract, A.max)
            nc.vector.tensor_scalar(cbase[:], cbase[:], float(W - WN), None, A.min)
            ry0 = pool.tile([P, T], F32)  # clip(py-2, 0, H-5)
            nc.vector.tensor_scalar(ry0[:], py, 2.0, 0.0, A.subtract, A.max)
            nc.vector.tensor_scalar(ry0[:], ry0[:], float(H - WN), None, A.min)

            ccp = pool.tile([P, T], F32)  # cbase - px  in [-2, 2]
            nc.vector.tensor_sub(ccp[:], cbase[:], px)
            rpy = pool.tile([P, T], F32)  # ry0 - py
            nc.vector.tensor_sub(rpy[:], ry0[:], py)

            # ---- gather indices ---------------------------------------
            # One index per (pair) = window origin; the HW indirect-DMA
            # unroll consumes exactly one index per destination partition
            # row and copies a contiguous run, so we fetch the whole
            # 4*W+5 = 1029-element span containing the 5x5 window; the
            # window then sits at static strides i*W+j inside the run.
            # idx[p, t] = (p + 128t)*H*W + ry0*W + cbase
            RUN = 4 * W + WN  # 1029
            PITCH = RUN + 3  # pad to multiple of 8 elements
            idxf = pool.tile([P, T], F32)
            idx = pool.tile([P, T], I32)
            # t=2 has only 16 live pairs: issue it last and only 16
            # partitions wide so the final transfer tail is tiny; the
            # dead region is zeroed early so downstream math stays finite.
            blk = pool.tile([P, T * PITCH], F32)
            nc.vector.memset(blk[:, 2 * PITCH :], 0)
            NPART = [P, P, 16]
            # chunk 0's index column is computed (and its gather launched)
            # before columns 1-2, overlapping SWDGE descgen with DVE work
            for cols in (slice(0, 1), slice(1, T)):
                nc.vector.scalar_tensor_tensor(
                    idxf[:, cols],
                    ry0[:, cols],
                    float(W),
                    cbase[:, cols],
                    op0=A.mult,
                    op1=A.add,
                )
                nc.vector.tensor_add(idxf[:, cols], idxf[:, cols], goff[:, cols])
                # clamp padding pairs (g >= 272) into bounds
                nc.vector.tensor_scalar(
                    idxf[:, cols], idxf[:, cols], float(NELEM - RUN), None, A.min
                )
                nc.vector.tensor_copy(idx[:, cols], idxf[:, cols])
                for t in range(cols.start, cols.stop):
                    nc.gpsimd.indirect_dma_start(
                        out=blk[: NPART[t], t * PITCH : t * PITCH + RUN],
                        out_offset=None,
                        in_=heat[:, :],
                        in_offset=bass.IndirectOffsetOnAxis(
                            ap=idx[: NPART[t], t : t + 1], axis=1
                        ),
                    )

            # ---- validity masks (additive -NEGM), all 2D APs ----------
            # drow75[p, 25t+5i+j] = (ry0 - py) + i ; dcol75 = (cbase - px) + j
            SS = WN * WN
            drow75 = pool.tile([P, T * SS], F32)
            dcol75 = pool.tile([P, T * SS], F32)
            for t in range(T):
                nc.vector.tensor_add(
                    drow75[:, SS * t : SS * (t + 1)],
                    _bcast_col(rpy[:], t, SS),
                    yoff[:, SS * t : SS * (t + 1)],
                )
                nc.vector.tensor_add(
                    dcol75[:, SS * t : SS * (t + 1)],
                    _bcast_col(ccp[:], t, SS),
                    xoff[:, SS * t : SS * (t + 1)],
                )
            # valid <=> |d| <= 2 <=> d*d <= 4.5 (d is integer-valued)
            rmask = pool.tile([P, T * SS], F32)  # 0 if valid else -NEGM
            nc.vector.tensor_mul(rmask[:], drow75[:], drow75[:])
            nc.vector.tensor_scalar(rmask[:], rmask[:], 4.5, None, A.is_le)
            nc.vector.tensor_scalar(rmask[:], rmask[:], 1.0, NEGM, A.subtract, A.mult)
            cmask = pool.tile([P, T * SS], F32)
            nc.vector.tensor_mul(cmask[:], dcol75[:], dcol75[:])
            nc.vector.tensor_scalar(cmask[:], cmask[:], 4.5, None, A.is_le)
            nc.vector.tensor_scalar(cmask[:], cmask[:], 1.0, NEGM, A.subtract, A.mult)

            nc.vector.tensor_add(rmask[:], rmask[:], cmask[:])

            # masked logits: window (i,j) of chunk t lives in blk at
            # offset t*PITCH + i*W + j  (static strides, cbase pre-folded)
            ml = pool.tile([P, T * SS], F32)
            bv = blk[:]
            mv = ml[:]
            rv_ = rmask[:]
            for t in range(T):
                win = bass.AP(
                    bv.tensor, bv.offset + t * PITCH, [bv.ap[0], [W, WN], [1, WN]]
                )
                nc.vector.tensor_add(
                    bass.AP(
                        mv.tensor, mv.offset + t * SS, [mv.ap[0], [WN, WN], [1, WN]]
                    ),
                    win,
                    bass.AP(
                        rv_.tensor, rv_.offset + t * SS, [rv_.ap[0], [WN, WN], [1, WN]]
                    ),
                )

            # ---- softmax moments --------------------------------------
            # logits are bounded (|heat|<6, masks >= -100) so exp() without
            # the max-shift is numerically safe and matches to ~1e-7 rel.
            ez = pool.tile([P, T * WN * WN], F32)
            nc.scalar.activation(ez[:], ml[:], mybir.ActivationFunctionType.Exp)

            ez3 = ez[:].rearrange("p (t s) -> p t s", s=WN * WN)
            ssum = pool.tile([P, T], F32)
            nc.vector.tensor_reduce(ssum[:], ez3, axis=mybir.AxisListType.X, op=A.add)
            rinv = pool.tile([P, T], F32)
            nc.vector.reciprocal(rinv[:], ssum[:])

            qx = pool.tile([P, T * WN * WN], F32)
            nc.vector.tensor_mul(qx[:], ez[:], xoff[:])
            qy = pool.tile([P, T * WN * WN], F32)
            nc.vector.tensor_mul(qy[:], ez[:], yoff[:])
            numx = pool.tile([P, T], F32)
            nc.vector.tensor_reduce(
                numx[:],
                qx[:].rearrange("p (t s) -> p t s", s=WN * WN),
                axis=mybir.AxisListType.X,
                op=A.add,
            )
            numy = pool.tile([P, T], F32)
            nc.vector.tensor_reduce(
                numy[:],
                qy[:].rearrange("p (t s) -> p t s", s=WN * WN),
                axis=mybir.AxisListType.X,
                op=A.add,
            )

            # rx = cbase + numx/ssum ; ry = ry0 + numy/ssum
            res = pool.tile([P, T * 2], F32)
            rv = res[:]
            rx_view = bass.AP(rv.tensor, rv.offset, [rv.ap[0], [2, T]])
            ry_view = bass.AP(rv.tensor, rv.offset + 1, [rv.ap[0], [2, T]])
            nc.vector.tensor_mul(numx[:], numx[:], rinv[:])
            nc.vector.tensor_add(rx_view, numx[:], cbase[:])
            nc.vector.tensor_mul(numy[:], numy[:], rinv[:])
            nc.vector.tensor_add(ry_view, numy[:], ry0[:])

            # ---- store ------------------------------------------------
            nc.sync.dma_start(
                out=out[:, :].rearrange("(t p) c -> p t c", p=P),
                in_=res[:].rearrange("p (t c) -> p t c", c=2),
            )
    nc.compile()
    return nc


_NC = None


def _get_nc():
    global _NC
    if _NC is None:
        _NC = build_program()
    return _NC


def make_in_maps(heatmaps: np.ndarray, coarse_coords: np.ndarray):
    heatmaps = np.ascontiguousarray(heatmaps, dtype=np.float32)
    coarse_coords = np.ascontiguousarray(coarse_coords, dtype=np.float32)
    in_maps = []
    for m in range(NCORES):
        hs = heatmaps[m * BS : (m + 1) * BS].reshape(R, W)
        cs = np.zeros((PADP, 2), dtype=np.float32)
        cs[:PAIRS] = coarse_coords[m * BS : (m + 1) * BS].reshape(PAIRS, 2)
        in_maps.append({"heat": hs, "coords": cs})
    return in_maps


def assemble_out(results) -> np.ndarray:
    outs = [results[m]["out"][:PAIRS].reshape(BS, K, 2) for m in range(NCORES)]
    return np.concatenate(outs, axis=0)


def kernel(heatmaps: np.ndarray, coarse_coords: np.ndarray) -> np.ndarray:
    nc = _get_nc()
    in_maps = make_in_maps(heatmaps, coarse_coords)
    results = run_bass_kernel_spmd(nc, in_maps, core_ids=list(range(NCORES)))
    return assemble_out(results.results)
